# revision 3
# baseline (speedup 1.0000x reference)
"""Chamfer distance kernel for 8 Trainium2 NeuronCores (Bass/Tile).

Problem: xyz1, xyz2: (4, 8192, 3) fp32. Outputs dist1, dist2: (4, 8192) fp32,
the row-wise / column-wise minima of the pairwise squared-distance matrix
d[n,m] = max(||x_n||^2 + ||y_m||^2 - 2 x_n.y_m, 0), per batch.

Sharding: core c handles batch c//2 and half of the N rows (c%2). Each core
computes dist1 for its 4096 rows exactly, and a dist2 partial (min over its
4096 rows) for all 8192 columns; the host min-combines the two partials.

Per-core kernel: distance tiles are produced by ONE bf16 matmul each, using
K=24 augmented vectors: bf16x3 decompositions of x, of -2*y and of the two
squared norms, ordered so the large terms cancel early in the fp32 PSUM
accumulation (fp32-faithful, representation residual ~2^-27).

Both outputs are row-min reductions of some orientation of d, so the kernel
computes BOTH orientations (d = xa^T ya and d^T = ya^T xa) into PSUM groups
of [128, grp x 512] (grp banks).

Reduction (the bottleneck: every distance element must be drained from PSUM,
and only the DVE and the Activation engine have PSUM read ports; a DVE
2-tensor op can read at most ONE input from PSUM; tensor_tensor_reduce
crashes this deployment's DVE, but tensor_tensor_scan is HW-verified):
  - 'A' groups: ScalarE activation(Relu) drains the group PSUM fp32 ->
    SBUF fp16 at 1.2 GHz; the converted tile is queued.
  - 'D' groups: ONE DVE tensor_tensor_scan with data0 = the group's PSUM
    banks and data1 = the oldest queued converted tile (SBUF port),
    op0=op1=min: state = min(data0[t], state, data1[t]). The DVE consumes
    both streams simultaneously (~2 elem/lane/cycle aggregate), and the
    scan's last output column is the running min of every group scanned so
    far -- scans chain across a weight tile via initial=prev[:, -1:], so
    ONE tiny copy per weight tile extracts the final min column.
The pattern per weight tile (a_pat/b_pat) staggers each A two groups ahead
of the D that rides it, hiding the Act->DVE latency; psum_bufs tiles of grp
banks ring-buffer so the PE runs ahead.

Final pass: relu only (the scan chain already did the full reduction).
"""

from contextlib import ExitStack

import numpy as np
import ml_dtypes

B, N, M = 4, 8192, 8192
NCORES = 8
NLOC = N // 2          # rows of xyz1 per core
P = 128                # partitions
FD = 512               # matmul free dim (one PSUM bank of fp32)
KAUG = 24
BIG = 3.0e38

_BF16 = ml_dtypes.bfloat16


def _decomp3(v):
    """fp32/fp64 array -> three bf16 planes summing to v (residual ~2^-27)."""
    v = v.astype(np.float32)
    h = v.astype(_BF16)
    r = v - h.astype(np.float32)
    m = r.astype(_BF16)
    r2 = r - m.astype(np.float32)
    l = r2.astype(_BF16)
    return h, m, l


def _build_aug(x, y):
    """x: [Nl,3] fp32, y: [Mm,3] fp32 -> (xa [KAUG,Nl] bf16, ya [KAUG,Mm] bf16).

    d[n,m] = sum_k xa[k,n]*ya[k,m] up to bf16x3 residuals. Slot order puts the
    large mutually-cancelling terms first so fp32 PSUM accumulation stays
    accurate near d ~ 0.
    """
    nl, mm = x.shape[0], y.shape[0]
    nx = (x.astype(np.float64) ** 2).sum(axis=1)
    ny = (y.astype(np.float64) ** 2).sum(axis=1)
    xh, xm, xl = _decomp3(x)
    y2 = (-2.0 * y.astype(np.float64)).astype(np.float32)
    yh, ym, yl = _decomp3(y2)
    nxh, nxm, nxl = _decomp3(nx)
    nyh, nym, nyl = _decomp3(ny)

    one_n = np.ones(nl, dtype=_BF16)
    one_m = np.ones(mm, dtype=_BF16)

    xa = np.empty((KAUG, nl), dtype=_BF16)
    ya = np.empty((KAUG, mm), dtype=_BF16)
    k = 0

    def slot(xv, yv):
        nonlocal k
        xa[k] = xv
        ya[k] = yv
        k += 1

    slot(nxh, one_m)
    slot(one_n, nyh)
    for c in range(3):
        slot(xh[:, c], yh[:, c])
    slot(nxm, one_m)
    slot(one_n, nym)
    for c in range(3):
        slot(xh[:, c], ym[:, c])
    for c in range(3):
        slot(xm[:, c], yh[:, c])
    slot(nxl, one_m)
    slot(one_n, nyl)
    for c in range(3):
        slot(xh[:, c], yl[:, c])
    for c in range(3):
        slot(xm[:, c], ym[:, c])
    for c in range(3):
        slot(xl[:, c], yh[:, c])
    assert k == KAUG
    return xa, ya


def build_bass(
    nloc=NLOC, m_total=M, repeat=1, grp=4, psum_bufs=2,
    a_pat="AADD", b_pat="AD", c_pad=16, c_bufs=6, cv_fp16=True,
):
    """Build + compile the per-core Bass program.

    repeat>1 wraps the main compute in a dynamic loop executing it `repeat`
    times — used only to measure per-iteration HW time above the PJRT
    dispatch noise floor.

    a_pat/b_pat: per-weight-tile routing over the groups of the A/B
    orientation ('A' = Act-convert & queue, 'D' = DVE TTR draining this
    group and riding the oldest queued converted tile). Equal counts of A
    and D per tile; every D needs an earlier A. c_pad pads the converted
    tiles' free dim (SBUF bank phase stagger); c_bufs sizes that pool.
    """
    import concourse.bacc as bacc
    import concourse.tile as tile
    import concourse.mybir as mybir

    f32 = mybir.dt.float32
    bf16 = mybir.dt.bfloat16
    cv_dt = mybir.dt.float16 if cv_fp16 else bf16
    Alu = mybir.AluOpType
    Act = mybir.ActivationFunctionType
    X = mybir.AxisListType.X

    ntile_a = nloc // P              # weight tiles, orientation A (dist1 rows)
    ngrp_a = m_total // (grp * FD)   # reduce groups per A weight tile
    ntile_b = m_total // P           # weight tiles, orientation B (dist2 rows)
    ngrp_b = nloc // (grp * FD)      # reduce groups per B weight tile

    assert len(a_pat) == ngrp_a and len(b_pat) == ngrp_b
    for p in (a_pat, b_pat):
        assert p.count("A") == p.count("D")
        assert set(p) <= {"A", "D", "R"}

    nc = bacc.Bacc("TRN2", target_bir_lowering=False, debug=False)
    xa_d = nc.dram_tensor("xa", [KAUG, nloc], bf16, kind="ExternalInput")
    ya_d = nc.dram_tensor("ya", [KAUG, m_total], bf16, kind="ExternalInput")
    d1_d = nc.dram_tensor("d1", [P, ntile_a], f32, kind="ExternalOutput")
    d2_d = nc.dram_tensor("d2", [P, ntile_b], f32, kind="ExternalOutput")

    with tile.TileContext(nc) as tc, ExitStack() as ctx:
        singles = ctx.enter_context(tc.tile_pool(name="singles", bufs=1))
        psum = ctx.enter_context(
            tc.tile_pool(name="psum", bufs=psum_bufs, space="PSUM")
        )

        # chunked loads so the first matmuls start before the full tensors land
        xa = singles.tile([KAUG, nloc], bf16)
        for i in range(4):
            sl = slice(i * nloc // 4, (i + 1) * nloc // 4)
            nc.sync.dma_start(out=xa[:, sl], in_=xa_d.ap()[:, sl])
        ya = singles.tile([KAUG, m_total], bf16)
        for i in range(4):
            sl = slice(i * m_total // 4, (i + 1) * m_total // 4)
            nc.sync.dma_start(out=ya[:, sl], in_=ya_d.ap()[:, sl])

        cpool = ctx.enter_context(tc.tile_pool(name="cpool", bufs=c_bufs))
        spool = ctx.enter_context(tc.tile_pool(name="spool", bufs=3))

        n_r_a, n_r_b = a_pat.count("R"), b_pat.count("R")
        # group-level column slots for 'R' groups, it-level slots for chains
        c1g = singles.tile([P, max(1, ntile_a * n_r_a)], f32)
        c2g = singles.tile([P, max(1, ntile_b * n_r_b)], f32)
        c1 = singles.tile([P, ntile_a], f32)
        c2 = singles.tile([P, ntile_b], f32)
        d1t = singles.tile([P, ntile_a], f32)
        d2t = singles.tile([P, ntile_b], f32)

        def orientation(wt_count, grp_count, w_sb, mv_sb, cols, colsg, n_r, pat):
            """One orientation: wt_count weight tiles x grp_count reduce
            groups; rows of the output come from w_sb, the reduction runs
            over all of mv_sb."""
            for it in range(wt_count):
                queue = []  # converted tiles awaiting a scan ride, FIFO
                prev = None  # previous scan's output tile (chain state)
                r_idx = 0

                for g in range(grp_count):
                    pt = psum.tile([P, grp, FD], f32, name="pt", tag="pt")
                    for j in range(grp):
                        nc.tensor.matmul(
                            pt[:, j, :],
                            w_sb[:, it * P : (it + 1) * P],
                            mv_sb[:, (g * grp + j) * FD : (g * grp + j + 1) * FD],
                            start=True,
                            stop=True,
                        )

                    if pat[g] == "R":
                        col = it * n_r + r_idx
                        r_idx += 1
                        nc.vector.tensor_reduce(
                            out=colsg[:, col : col + 1],
                            in_=pt.rearrange("p g f -> p (g f)"),
                            axis=X,
                            op=Alu.min,
                        )
                    elif pat[g] == "A":
                        # trailing pad staggers the SBUF bank phase between
                        # pool slots; the payload slice stays contiguous so
                        # the 2D scan view below is legal
                        cv = cpool.tile(
                            [P, grp * FD + c_pad], cv_dt, name="cv", tag="cv"
                        )
                        nc.scalar.activation(
                            out=cv[:, : grp * FD].rearrange(
                                "p (g f) -> p g f", g=grp
                            ),
                            in_=pt,
                            func=Act.Relu,
                        )
                        queue.append(cv)
                    else:  # 'D'
                        cv = queue.pop(0)
                        scr = spool.tile([P, grp * FD], f32, name="scr", tag="scr")
                        nc.vector.tensor_tensor_scan(
                            out=scr,
                            data0=pt.rearrange("p g f -> p (g f)"),
                            data1=cv[:, : grp * FD],
                            initial=BIG if prev is None else prev[:, -1:],
                            op0=Alu.min,
                            op1=Alu.min,
                        )
                        prev = scr
                assert not queue, "unconsumed converted groups in pattern"
                if prev is not None:
                    # ScalarE copy: DVE is the bottleneck engine, Act has slack
                    nc.scalar.copy(
                        out=cols[:, it : it + 1], in_=prev[:, -1:]
                    )

        def main_compute():
            orientation(ntile_a, ngrp_a, xa, ya, c1, c1g, n_r_a, a_pat)
            orientation(ntile_b, ngrp_b, ya, xa, c2, c2g, n_r_b, b_pat)

        if repeat == 1:
            main_compute()
        else:
            with tc.For_i(0, repeat, 1):
                main_compute()

        # finals: fold 'R' group columns (strided min) and any chain columns
        # into per-tile minima, then relu
        def finals(cols, colsg, n_r, ntile, has_chain, dt_out):
            if n_r:
                nc.vector.tensor_reduce(
                    out=dt_out,
                    in_=colsg[:, : ntile * n_r].rearrange(
                        "p (t g) -> p t g", g=n_r
                    ),
                    axis=X,
                    op=Alu.min,
                )
                if has_chain:
                    nc.vector.tensor_tensor(
                        out=dt_out, in0=dt_out, in1=cols, op=Alu.min
                    )
            else:
                nc.vector.tensor_copy(out=dt_out, in_=cols)
            nc.vector.tensor_scalar_max(out=dt_out, in0=dt_out, scalar1=0.0)

        finals(c1, c1g, n_r_a, ntile_a, "D" in a_pat, d1t)
        finals(c2, c2g, n_r_b, ntile_b, "D" in b_pat, d2t)

        nc.sync.dma_start(out=d1_d.ap(), in_=d1t)
        nc.sync.dma_start(out=d2_d.ap(), in_=d2t)

    nc.compile()
    return nc


_CACHED_NC = None


def _get_nc():
    global _CACHED_NC
    if _CACHED_NC is None:
        _CACHED_NC = build_bass()
    return _CACHED_NC


def _make_in_maps(xyz1, xyz2):
    xyz1 = np.asarray(xyz1, dtype=np.float32)
    xyz2 = np.asarray(xyz2, dtype=np.float32)
    in_maps = []
    for c in range(NCORES):
        b, h = divmod(c, 2)
        x = xyz1[b, h * NLOC : (h + 1) * NLOC]
        y = xyz2[b]
        xa, ya = _build_aug(x, y)
        in_maps.append({"xa": xa, "ya": ya})
    return in_maps


def _unshard(results):
    dist1 = np.empty((B, N), np.float32)
    dist2 = np.empty((B, M), np.float32)
    for c in range(NCORES):
        b, h = divmod(c, 2)
        dist1[b, h * NLOC : (h + 1) * NLOC] = np.asarray(results[c]["d1"]).T.ravel()
        d2p = np.asarray(results[c]["d2"]).T.ravel()
        if h == 0:
            dist2[b] = d2p
        else:
            np.minimum(dist2[b], d2p, out=dist2[b])
    return dist1, dist2


def kernel(xyz1, xyz2):
    from concourse.bass_utils import run_bass_kernel_spmd

    nc = _get_nc()
    in_maps = _make_in_maps(xyz1, xyz2)
    res = run_bass_kernel_spmd(nc, in_maps, core_ids=list(range(NCORES)))
    return _unshard(res.results)



# revision 7
# speedup vs baseline: 2.2061x; 2.2061x over previous
"""Chamfer distance kernel for 8 Trainium2 NeuronCores (Bass/Tile).

Problem: xyz1, xyz2: (4, 8192, 3) fp32. Outputs dist1, dist2: (4, 8192) fp32,
the row-wise / column-wise minima of the pairwise squared-distance matrix
d[n,m] = max(||x_n||^2 + ||y_m||^2 - 2 x_n.y_m, 0), per batch.

Sharding: core c handles batch c//2 and half of the N rows (c%2). Each core
computes dist1 for its 4096 rows exactly, and a dist2 partial (min over its
4096 rows) for all 8192 columns; the host min-combines the two partials.

Per-core kernel (ONE orientation only — d computed once):
  - distance tiles [128, FD] are produced by ONE bf16 matmul each, using
    K=24 augmented vectors: bf16x3 decompositions of x, of -2*y and of the
    two squared norms, ordered so the large terms cancel early in the fp32
    PSUM accumulation (fp32-faithful, representation residual ~2^-27).
  - HW-measured drain costs (FD=2048 cols of 128 lanes):
      DVE tensor_reduce (PSUM fp32)           ~2069 ns
      DVE tensor_tensor fp16 SBUF (2x mode)    ~927 ns
      ScalarE activation PSUM fp32 -> fp16    ~2287 ns
      DVE tensor_tensor_scan                  ~4500 ns  (NO dual-stream win)
    so the drain is split: ScalarE converts most groups to fp16 in SBUF
    (Relu fused), and the DVE does all min work on fp16 at 2x:
      dist1 (row-min): elementwise TT-min tree across the tile's groups,
        then halvings down to 64 cols; per-tile tails are folded by one
        final strided tensor_reduce.
      dist2 (col-min): acc_g = min(acc_g, cv_g) fp16 accumulators [128, M],
        partition-folded at the end by PE transposes (identity matmul)
        + small strided reduces of the transposed blocks.
    A few groups per 'direct_every' are drained straight from PSUM by the
    DVE (tensor_reduce row-min + TT col-acc) to balance ScalarE vs DVE.
"""

from contextlib import ExitStack

import numpy as np
import ml_dtypes

B, N, M = 4, 8192, 8192
NCORES = 8
NLOC = N // 2          # rows of xyz1 per core
P = 128                # partitions
FD = 512               # matmul free dim (one PSUM bank of fp32)
KAUG = 24
BIG = 3.0e38

_BF16 = ml_dtypes.bfloat16


def _decomp3(v):
    """fp32/fp64 array -> three bf16 planes summing to v (residual ~2^-27)."""
    v = v.astype(np.float32)
    h = v.astype(_BF16)
    r = v - h.astype(np.float32)
    m = r.astype(_BF16)
    r2 = r - m.astype(np.float32)
    l = r2.astype(_BF16)
    return h, m, l


def _build_aug(x, y):
    """x: [Nl,3] fp32, y: [Mm,3] fp32 -> (xa [KAUG,Nl] bf16, ya [KAUG,Mm] bf16).

    d[n,m] = sum_k xa[k,n]*ya[k,m] up to bf16x3 residuals. Slot order puts the
    large mutually-cancelling terms first so fp32 PSUM accumulation stays
    accurate near d ~ 0.
    """
    nl, mm = x.shape[0], y.shape[0]
    nx = (x.astype(np.float64) ** 2).sum(axis=1)
    ny = (y.astype(np.float64) ** 2).sum(axis=1)
    xh, xm, xl = _decomp3(x)
    y2 = (-2.0 * y.astype(np.float64)).astype(np.float32)
    yh, ym, yl = _decomp3(y2)
    nxh, nxm, nxl = _decomp3(nx)
    nyh, nym, nyl = _decomp3(ny)

    one_n = np.ones(nl, dtype=_BF16)
    one_m = np.ones(mm, dtype=_BF16)

    xa = np.empty((KAUG, nl), dtype=_BF16)
    ya = np.empty((KAUG, mm), dtype=_BF16)
    k = 0

    def slot(xv, yv):
        nonlocal k
        xa[k] = xv
        ya[k] = yv
        k += 1

    slot(nxh, one_m)
    slot(one_n, nyh)
    for c in range(3):
        slot(xh[:, c], yh[:, c])
    slot(nxm, one_m)
    slot(one_n, nym)
    for c in range(3):
        slot(xh[:, c], ym[:, c])
    for c in range(3):
        slot(xm[:, c], yh[:, c])
    slot(nxl, one_m)
    slot(one_n, nyl)
    for c in range(3):
        slot(xh[:, c], yl[:, c])
    for c in range(3):
        slot(xm[:, c], ym[:, c])
    for c in range(3):
        slot(xl[:, c], yh[:, c])
    assert k == KAUG
    return xa, ya


def build_bass(
    nloc=NLOC, m_total=M, repeat=1, grp=4, psum_bufs=2,
    c_bufs=6, c_pad=16, tail_stop=64, direct_every=16,
):
    """Build + compile the per-core Bass program.

    repeat>1 wraps the main compute in a dynamic loop executing it `repeat`
    times — used only to measure per-iteration HW time above the PJRT
    dispatch noise floor.

    direct_every: every direct_every-th (tile, group) is drained directly
    from PSUM by the DVE (reduce + TT col-acc) instead of being converted
    by ScalarE — load-balances the two drain engines. 0 disables.
    """
    import concourse.bacc as bacc
    import concourse.tile as tile
    import concourse.mybir as mybir

    f32 = mybir.dt.float32
    f16 = mybir.dt.float16
    bf16 = mybir.dt.bfloat16
    Alu = mybir.AluOpType
    Act = mybir.ActivationFunctionType
    X = mybir.AxisListType.X

    GFD = grp * FD                   # columns per reduce group
    ntile = nloc // P                # weight tiles (dist1 rows): 32
    ngrp = m_total // GFD            # reduce groups per weight tile: 4
    nblk = m_total // P              # dist2 output blocks: 64
    BIG16 = 6.0e4                    # > any distance, fp16-representable

    nc = bacc.Bacc("TRN2", target_bir_lowering=False, debug=False)
    xa_d = nc.dram_tensor("xa", [KAUG, nloc], bf16, kind="ExternalInput")
    ya_d = nc.dram_tensor("ya", [KAUG, m_total], bf16, kind="ExternalInput")
    id_d = nc.dram_tensor("ident", [P, P], f16, kind="ExternalInput")
    d1_d = nc.dram_tensor("d1", [P, ntile], f32, kind="ExternalOutput")
    d2_d = nc.dram_tensor("d2", [P, nblk], f32, kind="ExternalOutput")

    with tile.TileContext(nc) as tc, ExitStack() as ctx:
        singles = ctx.enter_context(tc.tile_pool(name="singles", bufs=1))
        psum = ctx.enter_context(
            tc.tile_pool(name="psum", bufs=psum_bufs, space="PSUM")
        )

        # chunked loads so the first matmuls start before the full tensors land
        xa = singles.tile([KAUG, nloc], bf16)
        for i in range(4):
            sl = slice(i * nloc // 4, (i + 1) * nloc // 4)
            nc.sync.dma_start(out=xa[:, sl], in_=xa_d.ap()[:, sl])
        ya = singles.tile([KAUG, m_total], bf16)
        for i in range(4):
            sl = slice(i * m_total // 4, (i + 1) * m_total // 4)
            nc.sync.dma_start(out=ya[:, sl], in_=ya_d.ap()[:, sl])
        ident = singles.tile([P, P], f16)
        nc.sync.dma_start(out=ident, in_=id_d.ap())

        cpool = ctx.enter_context(tc.tile_pool(name="cpool", bufs=c_bufs))
        tpool = ctx.enter_context(tc.tile_pool(name="tpool", bufs=2))

        # dist2 column-min accumulators, one per group: [P, GFD] fp16
        accs = [
            singles.tile([P, GFD], f16, name=f"acc{g}") for g in range(ngrp)
        ]
        # per-tile row-min tails (tail_stop cols each)
        tails = singles.tile([P, ntile * tail_stop], f16)
        # row-min results of 'direct' groups land here (BIG16 elsewhere)
        dcols_n = max(1, ntile * ngrp)
        dcols = singles.tile([P, dcols_n], f32)

        d1t = singles.tile([P, ntile], f32)
        d2t = singles.tile([P, nblk], f32)

        def main_compute():
            for a in accs:
                nc.vector.memset(a, BIG16)
            nc.vector.memset(dcols, BIG)

            gidx = 0
            for it in range(ntile):
                cvs = []
                for g in range(ngrp):
                    pt = psum.tile([P, grp, FD], f32, name="pt", tag="pt")
                    for j in range(grp):
                        nc.tensor.matmul(
                            pt[:, j, :],
                            xa[:, it * P : (it + 1) * P],
                            ya[:, (g * grp + j) * FD : (g * grp + j + 1) * FD],
                            start=True,
                            stop=True,
                        )
                    gidx += 1
                    direct = direct_every and (gidx % direct_every == 0)
                    if direct:
                        ptf = pt.rearrange("p g f -> p (g f)")
                        # row-min straight from PSUM
                        nc.vector.tensor_reduce(
                            out=dcols[:, gidx - 1 : gidx],
                            in_=ptf, axis=X, op=Alu.min,
                        )
                        # col-acc straight from PSUM
                        nc.vector.tensor_tensor(
                            out=accs[g], in0=ptf, in1=accs[g], op=Alu.min
                        )
                        cvs.append(None)
                    else:
                        cv = cpool.tile([P, GFD + c_pad], f16, name="cv", tag="cv")
                        nc.scalar.activation(
                            out=cv[:, :GFD].rearrange("p (g f) -> p g f", g=grp),
                            in_=pt,
                            func=Act.Relu,
                        )
                        # dist2 column accumulate (fp16 TT, 2x mode)
                        nc.vector.tensor_tensor(
                            out=accs[g], in0=cv[:, :GFD], in1=accs[g], op=Alu.min
                        )
                        cvs.append(cv)

                # dist1 row-min tree over this tile's converted groups
                live = [cv[:, :GFD] for cv in cvs if cv is not None]
                lvl = 0
                while len(live) > 1:
                    nxt = []
                    for i in range(0, len(live) - 1, 2):
                        w = int(live[i].shape[1])
                        o = tpool.tile([P, w], f16, name=f"tL{lvl}", tag=f"tL{lvl}")
                        nc.vector.tensor_tensor(
                            out=o, in0=live[i], in1=live[i + 1], op=Alu.min
                        )
                        nxt.append(o)
                    if len(live) % 2:
                        nxt.append(live[-1])
                    live = nxt
                    lvl += 1
                if live:
                    h = live[0]
                    w = int(h.shape[1])
                    while w > 2 * tail_stop:
                        o = tpool.tile([P, w // 2], f16, name=f"th{w}", tag=f"th{w}")
                        nc.vector.tensor_tensor(
                            out=o, in0=h[:, : w // 2], in1=h[:, w // 2 :],
                            op=Alu.min,
                        )
                        h, w = o, w // 2
                    ts = slice(it * tail_stop, (it + 1) * tail_stop)
                    nc.vector.tensor_tensor(
                        out=tails[:, ts], in0=h[:, :tail_stop],
                        in1=h[:, tail_stop:], op=Alu.min,
                    )
                else:  # all groups direct: nothing converted this tile
                    nc.vector.memset(tails[:, it * tail_stop:(it + 1) * tail_stop], BIG16)

            # ---- finals ----
            # dist1: fold tails + direct columns
            nc.vector.tensor_reduce(
                out=d1t,
                in_=tails.rearrange("p (t s) -> p t s", s=tail_stop),
                axis=X,
                op=Alu.min,
            )
            if direct_every:
                dmin = singles.tile([P, ntile], f32)
                nc.vector.tensor_reduce(
                    out=dmin,
                    in_=dcols.rearrange("p (t g) -> p t g", g=ngrp),
                    axis=X,
                    op=Alu.min,
                )
                nc.vector.tensor_tensor(out=d1t, in0=d1t, in1=dmin, op=Alu.min)
            nc.vector.tensor_scalar_max(out=d1t, in0=d1t, scalar1=0.0)

            # dist2: PE-transpose the accumulators, reduce transposed blocks.
            # 4 transposes land at the 4 bank starts of one pt-slot; one
            # strided reduce then folds all four [128,128] blocks.
            for g in range(ngrp):
                for t0 in range(0, GFD // P, 4):
                    pt = psum.tile([P, grp, FD], f32, name="pt", tag="pt")
                    nt = min(4, GFD // P - t0)
                    for j in range(nt):
                        tp = pt[:, j, : P // 2].bitcast(f16)
                        t = t0 + j
                        nc.tensor.transpose(
                            tp, accs[g][:, t * P : (t + 1) * P], ident
                        )
                    blk = g * (GFD // P) + t0
                    nc.vector.tensor_reduce(
                        out=d2t[:, blk : blk + nt],
                        in_=pt[:, :nt, : P // 2].bitcast(f16),
                        axis=X,
                        op=Alu.min,
                    )
            nc.vector.tensor_scalar_max(out=d2t, in0=d2t, scalar1=0.0)

        if repeat == 1:
            main_compute()
        else:
            with tc.For_i(0, repeat, 1):
                main_compute()

        nc.sync.dma_start(out=d1_d.ap(), in_=d1t)
        nc.sync.dma_start(out=d2_d.ap(), in_=d2t)

    nc.compile()
    return nc


_CACHED_NC = None


def _get_nc():
    global _CACHED_NC
    if _CACHED_NC is None:
        _CACHED_NC = build_bass()
    return _CACHED_NC


_IDENT = np.eye(P, dtype=np.float16)


def _make_in_maps(xyz1, xyz2):
    xyz1 = np.asarray(xyz1, dtype=np.float32)
    xyz2 = np.asarray(xyz2, dtype=np.float32)
    in_maps = []
    for c in range(NCORES):
        b, h = divmod(c, 2)
        x = xyz1[b, h * NLOC : (h + 1) * NLOC]
        y = xyz2[b]
        xa, ya = _build_aug(x, y)
        in_maps.append({"xa": xa, "ya": ya, "ident": _IDENT})
    return in_maps


def _unshard(results):
    dist1 = np.empty((B, N), np.float32)
    dist2 = np.empty((B, M), np.float32)
    for c in range(NCORES):
        b, h = divmod(c, 2)
        dist1[b, h * NLOC : (h + 1) * NLOC] = np.asarray(results[c]["d1"]).T.ravel()
        d2p = np.asarray(results[c]["d2"]).T.ravel()
        if h == 0:
            dist2[b] = d2p
        else:
            np.minimum(dist2[b], d2p, out=dist2[b])
    return dist1, dist2


def kernel(xyz1, xyz2):
    from concourse.bass_utils import run_bass_kernel_spmd

    nc = _get_nc()
    in_maps = _make_in_maps(xyz1, xyz2)
    res = run_bass_kernel_spmd(nc, in_maps, core_ids=list(range(NCORES)))
    return _unshard(res.results)
